# revision 5
# baseline (speedup 1.0000x reference)
"""CLUB loss kernel for 8x TRN2 NeuronCores.

Math: the reference computes, per sample b (L=512 positions, D=64 dims):
  mu     = MLP_mu(x)                [D, L]
  logvar = tanh(MLP_lv(x))          [D, L]
  iv     = exp(-logvar)
  positive[d,l] = -(mu - y)^2 * 0.5 * iv
  negative[d,l] = -mean_j (y[d,j] - mu[d,l])^2 * 0.5 * iv
  loss = mean over (b,l) of sum_d (positive - negative)

The pairwise LxL mean collapses using moments of y over positions:
  mean_j (y_j - mu)^2 = Ey2 - 2*mu*Ey + mu^2,  Ey = mean_j y_j, Ey2 = mean_j y_j^2
so with yd2 = 2*(y - Ey), ysq = y^2:
  positive - negative = 0.5*iv*(Ey2 - ysq + mu*yd2)
and loss = -0.5/(B*L) * sum_{b,d,l} [ ((ysq - Ey2) - mu*yd2) * iv ].

Sharding: data-parallel over batch B=8, one sample per core. Each core gets
its sample's x (as 2 row-chunks), y, and a packed weight image; it returns a
[64,1] per-partition partial sum. Host does the final tiny reduction.
"""

import sys

if "/opt/trn_rl_repo" not in sys.path:
    sys.path.insert(0, "/opt/trn_rl_repo")

import numpy as np

B, L = 8, 512
XD, YD, H = 192, 64, 128
NCORES = 8
# Weight pack columns: 2*128 (W1T row-chunk A's) + 2*128 (W1T chunk B's, each in
# partitions 64:128 to align with xb) + 2*64 (W2T) + 4 bias cols
WCOLS = 644

_CACHE: dict = {}


def build_nc(debug: bool = False):
    import concourse.bass as bass
    import concourse.bacc as bacc
    import concourse.tile as tile
    from concourse import mybir

    f32 = mybir.dt.float32
    AF = mybir.ActivationFunctionType
    OP = mybir.AluOpType

    nc = bacc.Bacc("TRN2", target_bir_lowering=False, debug=debug)

    xa_d = nc.dram_tensor("xa", [128, L], f32, kind="ExternalInput")
    xby_d = nc.dram_tensor("xby", [128, L], f32, kind="ExternalInput")
    wp_d = nc.dram_tensor("wp", [128, WCOLS], f32, kind="ExternalInput")
    acc_d = nc.dram_tensor("acc", [YD, 1], f32, kind="ExternalOutput")

    with tile.TileContext(nc) as tc:
        with (
            tc.tile_pool(name="sb", bufs=1) as sb,
            tc.tile_pool(name="ps", bufs=1, space=bass.MemorySpace.PSUM) as ps,
        ):
            wp = sb.tile([128, WCOLS], f32, tag="wp")
            nc.sync.dma_start(out=wp, in_=wp_d[:, :])
            xa = sb.tile([128, L], f32, tag="xa")
            nc.sync.dma_start(out=xa, in_=xa_d[:, :])
            xby = sb.tile([128, L], f32, tag="xby")
            nc.sync.dma_start(out=xby, in_=xby_d[:, :])

            y = xby[0:64, :]
            xb = xby[64:128, :]

            # weight slices (packed by host; see kernel() below)
            w1muT_a = wp[:, 0:128]         # mu_W1.T rows 0:128
            w1lvT_a = wp[:, 128:256]       # lv_W1.T rows 0:128
            w1muT_b = wp[64:128, 256:384]  # mu_W1.T rows 128:192
            w1lvT_b = wp[64:128, 384:512]  # lv_W1.T rows 128:192
            w2muT = wp[:, 512:576]
            w2lvT = wp[:, 576:640]
            b1mu = wp[:, 640:641]
            b1lv = wp[:, 641:642]
            b2mu = wp[0:64, 642:643]
            b2lv = wp[0:64, 643:644]

            # --- moments of y over L ---
            sumy = sb.tile([64, 1], f32, tag="sumy")
            nc.vector.reduce_sum(out=sumy, in_=y, axis=mybir.AxisListType.X)
            ey = sb.tile([64, 1], f32, tag="ey")
            nc.scalar.mul(ey, sumy, 1.0 / L)
            ysq = sb.tile([64, L], f32, tag="ysq")
            sumy2 = sb.tile([64, 1], f32, tag="sumy2")
            nc.scalar.activation(out=ysq, in_=y, func=AF.Square, accum_out=sumy2)
            ey2 = sb.tile([64, 1], f32, tag="ey2")
            nc.scalar.mul(ey2, sumy2, 1.0 / L)

            # --- mu path ---
            h_mu = ps.tile([128, L], f32, tag="hmu")
            nc.tensor.matmul(h_mu, w1muT_a, xa, start=True, stop=False)
            nc.tensor.matmul(h_mu, w1muT_b, xb, start=False, stop=True)
            h_mu_s = sb.tile([128, L], f32, tag="hmus")
            nc.scalar.activation(out=h_mu_s, in_=h_mu, func=AF.Relu, bias=b1mu, scale=1.0)
            mu_nb = ps.tile([64, L], f32, tag="munb")
            nc.tensor.matmul(mu_nb, w2muT, h_mu_s, start=True, stop=True)

            # --- lv path ---
            h_lv = ps.tile([128, L], f32, tag="hlv")
            nc.tensor.matmul(h_lv, w1lvT_a, xa, start=True, stop=False)
            nc.tensor.matmul(h_lv, w1lvT_b, xb, start=False, stop=True)
            h_lv_s = sb.tile([128, L], f32, tag="hlvs")
            nc.scalar.activation(out=h_lv_s, in_=h_lv, func=AF.Relu, bias=b1lv, scale=1.0)
            lv_nb = ps.tile([64, L], f32, tag="lvnb")
            nc.tensor.matmul(lv_nb, w2lvT, h_lv_s, start=True, stop=True)
            t1 = sb.tile([64, L], f32, tag="t1")
            nc.scalar.activation(out=t1, in_=lv_nb, func=AF.Tanh, bias=b2lv, scale=1.0)
            iv = sb.tile([64, L], f32, tag="iv")
            nc.scalar.activation(out=iv, in_=t1, func=AF.Exp, scale=-1.0)

            # --- elementwise + reduction ---
            yd2 = sb.tile([64, L], f32, tag="yd2")
            nc.vector.tensor_scalar(
                out=yd2, in0=y, scalar1=ey, scalar2=2.0, op0=OP.subtract, op1=OP.mult
            )
            w = sb.tile([64, L], f32, tag="w")
            nc.vector.scalar_tensor_tensor(
                out=w, in0=mu_nb, scalar=b2mu, in1=yd2, op0=OP.add, op1=OP.mult
            )
            v = sb.tile([64, L], f32, tag="v")
            nc.vector.scalar_tensor_tensor(
                out=v, in0=ysq, scalar=ey2, in1=w, op0=OP.subtract, op1=OP.subtract
            )
            scr = sb.tile([64, L], f32, tag="scr")
            acc_s = sb.tile([64, 1], f32, tag="accs")
            nc.vector.scalar_tensor_tensor(
                out=scr,
                in0=v,
                scalar=1.0,
                in1=iv,
                op0=OP.mult,
                op1=OP.mult,
                accum_out=acc_s,
            )
            nc.sync.dma_start(out=acc_d[:, :], in_=acc_s)

    nc.compile()
    return nc


def pack_inputs(inputs: dict) -> list[dict]:
    x = np.ascontiguousarray(np.asarray(inputs["x_samples"], dtype=np.float32))
    y = np.ascontiguousarray(np.asarray(inputs["y_samples"], dtype=np.float32))
    mu_W1 = np.asarray(inputs["mu_W1"], dtype=np.float32)
    mu_b1 = np.asarray(inputs["mu_b1"], dtype=np.float32)
    mu_W2 = np.asarray(inputs["mu_W2"], dtype=np.float32)
    mu_b2 = np.asarray(inputs["mu_b2"], dtype=np.float32)
    lv_W1 = np.asarray(inputs["lv_W1"], dtype=np.float32)
    lv_b1 = np.asarray(inputs["lv_b1"], dtype=np.float32)
    lv_W2 = np.asarray(inputs["lv_W2"], dtype=np.float32)
    lv_b2 = np.asarray(inputs["lv_b2"], dtype=np.float32)

    wp = np.zeros((128, WCOLS), np.float32)
    w1muT = mu_W1.T  # [192, 128]
    w1lvT = lv_W1.T
    wp[:, 0:128] = w1muT[0:128]
    wp[:, 128:256] = w1lvT[0:128]
    wp[64:128, 256:384] = w1muT[128:192]
    wp[64:128, 384:512] = w1lvT[128:192]
    wp[:, 512:576] = mu_W2.T
    wp[:, 576:640] = lv_W2.T
    wp[:, 640] = mu_b1
    wp[:, 641] = lv_b1
    wp[0:64, 642] = mu_b2
    wp[0:64, 643] = lv_b2

    in_maps = []
    for b in range(NCORES):
        xby = np.empty((128, L), np.float32)
        xby[0:64] = y[b]
        xby[64:128] = x[b, 128:192]
        in_maps.append(
            {
                "xa": np.ascontiguousarray(x[b, 0:128]),
                "xby": xby,
                "wp": wp,
            }
        )
    return in_maps


def kernel(**inputs) -> np.ndarray:
    from concourse.bass_utils import run_bass_kernel_spmd

    if "nc" not in _CACHE:
        _CACHE["nc"] = build_nc(debug=False)
    nc = _CACHE["nc"]

    in_maps = pack_inputs(inputs)
    res = run_bass_kernel_spmd(nc, in_maps, core_ids=list(range(NCORES)))
    tot = 0.0
    for r in res.results:
        tot += float(r["acc"].astype(np.float64).sum())
    loss = -0.5 * tot / (B * L)
    return np.array(loss, dtype=np.float32)


# revision 8
# speedup vs baseline: 1.2293x; 1.2293x over previous
"""CLUB loss kernel for 8x TRN2 NeuronCores.

Math: the reference computes, per sample b (L=512 positions, D=64 dims):
  mu     = MLP_mu(x)                [D, L]
  logvar = tanh(MLP_lv(x))          [D, L]
  iv     = exp(-logvar)
  positive[d,l] = -(mu - y)^2 * 0.5 * iv
  negative[d,l] = -mean_j (y[d,j] - mu[d,l])^2 * 0.5 * iv
  loss = mean over (b,l) of sum_d (positive - negative)

The pairwise LxL mean collapses using moments of y over positions:
  mean_j (y_j - mu)^2 = Ey2 - 2*mu*Ey + mu^2,  Ey = mean_j y_j, Ey2 = mean_j y_j^2
so with yd2 = 2*(y - Ey), ysq = y^2:
  positive - negative = 0.5*iv*(Ey2 - ysq + mu*yd2)
and loss = -0.5/(B*L) * sum_{b,d,l} [ ((ysq - Ey2) - mu*yd2) * iv ].

Sharding: data-parallel over batch B=8, one sample per core. Each core gets
its sample's x, y, and a packed weight image; it returns a [64,1]
per-partition partial sum. Host does the final tiny reduction.

Matmuls run in float32r (fp32 bits, single-pass PE mode, ~4x faster than
fp32); y, biases and all elementwise math stay fp32.
"""

import sys

if "/opt/trn_rl_repo" not in sys.path:
    sys.path.insert(0, "/opt/trn_rl_repo")

import numpy as np

B, L = 8, 512
XD, YD, H = 192, 64, 128
NCORES = 8
# Weight pack columns (float32r): 2*128 W1T row-chunk A's + 2*128 W1T chunk B's
# (each in partitions 64:128, aligned with xb) + 2*64 W2T
WCOLS = 640

_CACHE: dict = {}


def build_nc(debug: bool = False):
    import concourse.bass as bass
    import concourse.bacc as bacc
    import concourse.tile as tile
    from concourse import mybir

    f32 = mybir.dt.float32
    f32r = mybir.dt.float32r
    AF = mybir.ActivationFunctionType
    OP = mybir.AluOpType

    nc = bacc.Bacc("TRN2", target_bir_lowering=False, debug=debug)

    xa_d = nc.dram_tensor("xa", [128, L], f32r, kind="ExternalInput")
    xb_d = nc.dram_tensor("xb", [64, L], f32r, kind="ExternalInput")
    y_d = nc.dram_tensor("y", [64, L], f32, kind="ExternalInput")
    wp_d = nc.dram_tensor("wp", [128, WCOLS], f32r, kind="ExternalInput")
    bias_d = nc.dram_tensor("bias", [128, 4], f32, kind="ExternalInput")
    acc_d = nc.dram_tensor("acc", [YD, 1], f32, kind="ExternalOutput")

    with tile.TileContext(nc) as tc:
        with (
            tc.tile_pool(name="sb", bufs=1) as sb,
            tc.tile_pool(name="ps", bufs=1, space=bass.MemorySpace.PSUM) as ps,
        ):
            wp = sb.tile([128, WCOLS], f32r, tag="wp")
            nc.sync.dma_start(out=wp, in_=wp_d[:, :])
            xa = sb.tile([128, L], f32r, tag="xa")
            nc.sync.dma_start(out=xa, in_=xa_d[:, :])
            xbr = sb.tile([128, L], f32r, tag="xbr")
            nc.sync.dma_start(out=xbr[64:128, :], in_=xb_d[:, :])
            yt = sb.tile([64, L], f32, tag="yt")
            nc.sync.dma_start(out=yt, in_=y_d[:, :])
            bias = sb.tile([128, 4], f32, tag="bias")
            nc.sync.dma_start(out=bias, in_=bias_d[:, :])

            y = yt[:, :]
            xb = xbr[64:128, :]

            w1muT_a = wp[:, 0:128]         # mu_W1.T rows 0:128
            w1lvT_a = wp[:, 128:256]       # lv_W1.T rows 0:128
            w1muT_b = wp[64:128, 256:384]  # mu_W1.T rows 128:192
            w1lvT_b = wp[64:128, 384:512]  # lv_W1.T rows 128:192
            w2muT = wp[:, 512:576]
            w2lvT = wp[:, 576:640]
            b1mu = bias[:, 0:1]
            b1lv = bias[:, 1:2]
            b2mu = bias[0:64, 2:3]
            b2lv = bias[0:64, 3:4]

            # --- moments of y over L ---
            sumy = sb.tile([64, 1], f32, tag="sumy")
            nc.vector.reduce_sum(out=sumy, in_=y, axis=mybir.AxisListType.X)
            ey = sb.tile([64, 1], f32, tag="ey")
            nc.scalar.mul(ey, sumy, 1.0 / L)
            ysq = sb.tile([64, L], f32, tag="ysq")
            sumy2 = sb.tile([64, 1], f32, tag="sumy2")
            nc.scalar.activation(out=ysq, in_=y, func=AF.Square, accum_out=sumy2)
            ey2 = sb.tile([64, 1], f32, tag="ey2")
            nc.scalar.mul(ey2, sumy2, 1.0 / L)

            # --- mu path ---
            h_mu = ps.tile([128, L], f32, tag="hmu")
            nc.tensor.matmul(h_mu, w1muT_a, xa[:, :], start=True, stop=False)
            nc.tensor.matmul(h_mu, w1muT_b, xb, start=False, stop=True)
            h_mu_s = sb.tile([128, L], f32r, tag="hmus")
            nc.scalar.activation(out=h_mu_s, in_=h_mu, func=AF.Relu, bias=b1mu, scale=1.0)
            mu_nb = ps.tile([64, L], f32, tag="munb")
            nc.tensor.matmul(mu_nb, w2muT, h_mu_s[:, :], start=True, stop=True)

            # --- lv path ---
            h_lv = ps.tile([128, L], f32, tag="hlv")
            nc.tensor.matmul(h_lv, w1lvT_a, xa[:, :], start=True, stop=False)
            nc.tensor.matmul(h_lv, w1lvT_b, xb, start=False, stop=True)
            h_lv_s = sb.tile([128, L], f32r, tag="hlvs")
            nc.scalar.activation(out=h_lv_s, in_=h_lv, func=AF.Relu, bias=b1lv, scale=1.0)
            lv_nb = ps.tile([64, L], f32, tag="lvnb")
            nc.tensor.matmul(lv_nb, w2lvT, h_lv_s[:, :], start=True, stop=True)
            t1 = sb.tile([64, L], f32, tag="t1")
            nc.scalar.activation(out=t1, in_=lv_nb, func=AF.Tanh, bias=b2lv, scale=1.0)
            iv = sb.tile([64, L], f32, tag="iv")
            nc.scalar.activation(out=iv, in_=t1, func=AF.Exp, scale=-1.0)

            # --- elementwise + reduction ---
            yd2 = sb.tile([64, L], f32, tag="yd2")
            nc.vector.tensor_scalar(
                out=yd2, in0=y, scalar1=ey, scalar2=2.0, op0=OP.subtract, op1=OP.mult
            )
            w = sb.tile([64, L], f32, tag="w")
            nc.vector.scalar_tensor_tensor(
                out=w, in0=mu_nb, scalar=b2mu, in1=yd2, op0=OP.add, op1=OP.mult
            )
            v = sb.tile([64, L], f32, tag="v")
            nc.vector.scalar_tensor_tensor(
                out=v, in0=ysq, scalar=ey2, in1=w, op0=OP.subtract, op1=OP.subtract
            )
            scr = sb.tile([64, L], f32, tag="scr")
            acc_s = sb.tile([64, 1], f32, tag="accs")
            nc.vector.scalar_tensor_tensor(
                out=scr,
                in0=v,
                scalar=1.0,
                in1=iv,
                op0=OP.mult,
                op1=OP.mult,
                accum_out=acc_s,
            )
            nc.sync.dma_start(out=acc_d[:, :], in_=acc_s)

    nc.compile()
    return nc


def pack_inputs(inputs: dict) -> list[dict]:
    x = np.ascontiguousarray(np.asarray(inputs["x_samples"], dtype=np.float32))
    y = np.ascontiguousarray(np.asarray(inputs["y_samples"], dtype=np.float32))
    mu_W1 = np.asarray(inputs["mu_W1"], dtype=np.float32)
    mu_b1 = np.asarray(inputs["mu_b1"], dtype=np.float32)
    mu_W2 = np.asarray(inputs["mu_W2"], dtype=np.float32)
    mu_b2 = np.asarray(inputs["mu_b2"], dtype=np.float32)
    lv_W1 = np.asarray(inputs["lv_W1"], dtype=np.float32)
    lv_b1 = np.asarray(inputs["lv_b1"], dtype=np.float32)
    lv_W2 = np.asarray(inputs["lv_W2"], dtype=np.float32)
    lv_b2 = np.asarray(inputs["lv_b2"], dtype=np.float32)

    wp = np.zeros((128, WCOLS), np.float32)
    w1muT = mu_W1.T  # [192, 128]
    w1lvT = lv_W1.T
    wp[:, 0:128] = w1muT[0:128]
    wp[:, 128:256] = w1lvT[0:128]
    wp[64:128, 256:384] = w1muT[128:192]
    wp[64:128, 384:512] = w1lvT[128:192]
    wp[:, 512:576] = mu_W2.T
    wp[:, 576:640] = lv_W2.T

    bias = np.zeros((128, 4), np.float32)
    bias[:, 0] = mu_b1
    bias[:, 1] = lv_b1
    bias[0:64, 2] = mu_b2
    bias[0:64, 3] = lv_b2

    in_maps = []
    for b in range(NCORES):
        in_maps.append(
            {
                "xa": np.ascontiguousarray(x[b, 0:128]),
                "xb": np.ascontiguousarray(x[b, 128:192]),
                "y": y[b],
                "wp": wp,
                "bias": bias,
            }
        )
    return in_maps


def kernel(**inputs) -> np.ndarray:
    from concourse.bass_utils import run_bass_kernel_spmd

    if "nc" not in _CACHE:
        _CACHE["nc"] = build_nc(debug=False)
    nc = _CACHE["nc"]

    in_maps = pack_inputs(inputs)
    res = run_bass_kernel_spmd(nc, in_maps, core_ids=list(range(NCORES)))
    tot = 0.0
    for r in res.results:
        tot += float(r["acc"].astype(np.float64).sum())
    loss = -0.5 * tot / (B * L)
    return np.array(loss, dtype=np.float32)
